# revision 13
# baseline (speedup 1.0000x reference)
"""Trainium2 Bass kernel for GQA MultiHeadAttention (nn_MultiHeadAttention_74028056314029).

Reference computation (fp32, single device):
    Q = x @ W_q.T         [C, H*D]   (H=32 query heads)
    K = x @ W_k.T         [C, KV*D]  (KV=8 kv heads, GQA group G=4)
    V = x @ W_v.T
    per query head: softmax(causal(Q_h K_h^T / sqrt(D))) @ V_h
    out = hidden @ W_o.T  [C, E]

Sharding (8 NeuronCores, tensor-parallel over heads):
    core c owns query heads [4c, 4c+4) == KV group c (1 KV head).
    W_q/W_o are split on the H*D dim, W_k/W_v on the KV*D dim.
    Each core computes a full-shape partial of the output projection;
    the partials are summed on the host (no on-device collective needed).

Device layouts (everything bf16 except PSUM accumulation / final output):
    x.T, W slices are pre-transposed on the host so all DMAs are contiguous.
    Q.T/K.T [d, seq] with head-dim on partitions feeds the scores matmul
    scores.T[s, q]; exp runs on ACT (scale=1/8 fused); attn@V uses V with an
    appended ones-column so the softmax denominator falls out of the same
    matmul (row 64 of the PSUM accumulator).
"""

import os
import numpy as np
import ml_dtypes

E, H, KVH, D = 2048, 32, 8, 64
B, C = 1, 2048
G = H // KVH              # 4 query heads per core
NCORES = 8
HD_C = G * D              # 256 query head dims per core
P = 128
NE = E // P               # 16 contraction chunks
NQ = C // P               # 16 sequence chunks
SW = 512                  # strip width (one PSUM bank of fp32)
NS = C // SW              # 4 strips

BF16 = ml_dtypes.bfloat16

_CACHE: dict = {}
LAST_RESULTS = None       # BassKernelResults of the most recent run (for profiling)
TRACE = bool(int(os.environ.get("KERNEL_TRACE", "0")))


def build_bass():
    import concourse.tile as tile
    import concourse.mybir as mybir
    from concourse import bacc
    from concourse.masks import make_identity

    bf16 = mybir.dt.bfloat16
    f32 = mybir.dt.float32
    AF = mybir.ActivationFunctionType

    nc = bacc.Bacc()
    xT = nc.declare_dram_parameter("xT", [E, C], bf16, isOutput=False)
    wqT = nc.declare_dram_parameter("wqT", [E, HD_C], bf16, isOutput=False)
    wkvT = nc.declare_dram_parameter("wkvT", [E, 2 * D], bf16, isOutput=False)
    woT = nc.declare_dram_parameter("woT", [HD_C, E], bf16, isOutput=False)
    tri = nc.declare_dram_parameter("tri", [P, P], bf16, isOutput=False)
    outp = nc.declare_dram_parameter("out_part", [C, E], f32, isOutput=True)

    with tile.TileContext(nc) as tc:
        with (
            tc.tile_pool(name="big", bufs=1) as big,
            tc.tile_pool(name="expp", bufs=4) as expp,
            tc.tile_pool(name="norm", bufs=2) as normp,
            tc.tile_pool(name="outs", bufs=2) as outs,
            tc.tile_pool(name="ps", bufs=2, space="PSUM") as ps,
            tc.tile_pool(name="pst", bufs=1, space="PSUM") as pst,
            tc.tile_pool(name="psb", bufs=1, space="PSUM") as psb,
            tc.tile_pool(name="psh", bufs=1, space="PSUM") as psh,
        ):
            # ---- persistent SBUF tensors ----
            x_sb = big.tile([P, NE, C], bf16)        # x.T: E on partitions
            wq_sb = big.tile([P, NE, HD_C], bf16)
            wkv_sb = big.tile([P, NE, 2 * D], bf16)  # [W_k | W_v] slice, transposed
            wo_sb = big.tile([P, 2, E], bf16)        # W_o slice transposed: hd on partitions
            tri_sb = big.tile([P, P], bf16)          # upper-tri ones (s<=q valid)
            ident = big.tile([P, P], bf16)
            ones_sb = big.tile([P, D], f32)          # ones row for PE broadcast
            kt_sb = big.tile([P, C], bf16)           # K.T duplicated on both halves
            vt_sb = big.tile([P, C], bf16)           # V.T staged at partitions 64:128
            v_sb = big.tile([P, NQ, D + 1], bf16)    # V natural + ones column
            qt_sb = big.tile([P, 2, C], bf16)        # Q.T: head-dim on partitions
            hid_sb = big.tile([P, 2, C], bf16)       # normalized hidden.T

            nc.sync.dma_start(out=x_sb, in_=xT[:].rearrange("(eo p) c -> p eo c", p=P))
            nc.sync.dma_start(out=wq_sb, in_=wqT[:].rearrange("(eo p) m -> p eo m", p=P))
            nc.sync.dma_start(out=wkv_sb, in_=wkvT[:].rearrange("(eo p) m -> p eo m", p=P))
            nc.sync.dma_start(out=wo_sb, in_=woT[:].rearrange("(ho p) e -> p ho e", p=P))
            nc.sync.dma_start(out=tri_sb, in_=tri[:])
            make_identity(nc, ident)
            nc.vector.memset(v_sb, 1.0)   # ones column survives; V copies overwrite the rest
            nc.vector.memset(ones_sb, 1.0)

            # ---- K/V projection: psum = [K.T ; V.T] (kv head on partitions) ----
            for s in range(NS):
                pkv = ps.tile([P, SW], f32, tag="mm")
                for eo in range(NE):
                    nc.tensor.matmul(
                        pkv, wkv_sb[:, eo, :], x_sb[:, eo, s * SW:(s + 1) * SW],
                        start=(eo == 0), stop=(eo == NE - 1))
                nc.vector.tensor_copy(out=kt_sb[0:D, s * SW:(s + 1) * SW], in_=pkv[0:D, :])
                nc.vector.tensor_copy(out=vt_sb[D:P, s * SW:(s + 1) * SW], in_=pkv[D:P, :])
            # duplicate K.T to partitions 64:128 (for row-tiled odd heads)
            nc.sync.dma_start(out=kt_sb[D:P, :], in_=kt_sb[0:D, :])
            # transpose V.T -> V natural [s, d] chunks
            for i in range(NQ):
                ptr = pst.tile([P, D], bf16, tag="tr")
                nc.tensor.transpose(ptr, vt_sb[D:P, i * P:(i + 1) * P], ident[D:P, D:P])
                nc.vector.tensor_copy(out=v_sb[:, i, 0:D], in_=ptr)

            # ---- Q projection (Q.T layout: head dim on partitions) ----
            for m in range(2):
                for s in range(NS):
                    pq = ps.tile([P, SW], f32, tag="mm")
                    for eo in range(NE):
                        nc.tensor.matmul(
                            pq, wq_sb[:, eo, m * P:(m + 1) * P],
                            x_sb[:, eo, s * SW:(s + 1) * SW],
                            start=(eo == 0), stop=(eo == NE - 1))
                    nc.vector.tensor_copy(out=qt_sb[:, m, s * SW:(s + 1) * SW], in_=pq)

            # ---- attention, one query head at a time ----
            for h in range(G):
                m, poff = h // 2, D * (h % 2)
                ph = psh.tile([D + 1, C], f32, tag="hid")   # hidden.T rows + den row
                for j in range(NS):                         # output q strip
                    for i in range(4 * j + 4):              # causal s-chunks
                        qlo = max(SW * j, P * i)
                        qhi = SW * (j + 1)
                        w = qhi - qlo
                        psc = ps.tile([P, SW], f32, tag="mm")
                        nc.tensor.matmul(
                            psc[:, :w],
                            kt_sb[poff:poff + D, i * P:(i + 1) * P],
                            qt_sb[poff:poff + D, m, qlo:qhi],
                            start=True, stop=True)
                        et = expp.tile([P, SW], bf16, tag="exp")
                        nc.scalar.activation(out=et[:, :w], in_=psc[:, :w],
                                             func=AF.Exp, scale=0.125)
                        if qlo == P * i:   # diagonal block: zero invalid (q<s) entries
                            nc.vector.tensor_mul(et[:, 0:P], et[:, 0:P], tri_sb)
                        nc.tensor.matmul(
                            ph[:, qlo:qhi], v_sb[:, i, :], et[:, :w],
                            start=(i == 0), stop=(i == 4 * j + 3),
                            skip_group_check=True)
                # normalize hidden.T by the fused denominator row: the PE
                # broadcasts 1/den across partitions via a K=1 ones matmul.
                for s in range(NS):
                    sl = slice(s * SW, (s + 1) * SW)
                    rec = normp.tile([P, SW], f32, tag="rec")
                    nc.vector.reciprocal(out=rec[D:D + 1, :], in_=ph[D:D + 1, sl])
                    pb = psb.tile([D, SW], f32, tag="bc")
                    nc.tensor.matmul(
                        pb, ones_sb[D:D + 1, :], rec[D:D + 1, :],
                        start=True, stop=True)
                    pb_sb = normp.tile([D, SW], f32, tag="pbs")
                    nc.vector.tensor_copy(out=pb_sb, in_=pb)
                    if poff == 0:
                        nc.vector.tensor_mul(hid_sb[0:D, m, sl], ph[0:D, sl], pb_sb)
                    else:
                        ht = normp.tile([D, SW], bf16, tag="ht")
                        nc.vector.tensor_mul(ht, ph[0:D, sl], pb_sb)
                        nc.sync.dma_start(out=hid_sb[D:P, m, sl], in_=ht)

            # ---- output projection partial: out[q, e] = hidden @ W_o_c.T ----
            for qc in range(NQ):
                o_sb = outs.tile([P, E], f32, tag="o")
                for es in range(NS):
                    po = ps.tile([P, SW], f32, tag="mm")
                    for m in range(2):
                        nc.tensor.matmul(
                            po, hid_sb[:, m, qc * P:(qc + 1) * P],
                            wo_sb[:, m, es * SW:(es + 1) * SW],
                            start=(m == 0), stop=(m == 1))
                    nc.vector.tensor_copy(out=o_sb[:, es * SW:(es + 1) * SW], in_=po)
                nc.sync.dma_start(out=outp[qc * P:(qc + 1) * P, :], in_=o_sb)

    nc.finalize()
    return nc


def make_core_inputs(x, W_q, W_k, W_v, W_o):
    """Host-side shard + pre-transpose + bf16 cast. Returns list of in_maps."""
    x2 = np.ascontiguousarray(x.reshape(C, E).T).astype(BF16)      # [E, C]
    tri_np = np.triu(np.ones((P, P), np.float32)).astype(BF16)     # q>=s valid
    in_maps = []
    for c in range(NCORES):
        qsl = slice(c * HD_C, (c + 1) * HD_C)
        ksl = slice(c * D, (c + 1) * D)
        wq_t = np.ascontiguousarray(W_q[qsl].T).astype(BF16)                    # [E, 256]
        wkv = np.concatenate([W_k[ksl], W_v[ksl]], axis=0)                      # [128, E]
        wkv_t = np.ascontiguousarray(wkv.T).astype(BF16)                        # [E, 128]
        wo_t = np.ascontiguousarray(W_o[:, qsl].T).astype(BF16)                 # [256, E]
        in_maps.append({
            "xT": x2, "wqT": wq_t, "wkvT": wkv_t, "woT": wo_t, "tri": tri_np,
        })
    return in_maps


def kernel(x, W_q, W_k, W_v, W_o):
    global LAST_RESULTS
    from concourse.bass_utils import run_bass_kernel_spmd

    if "nc" not in _CACHE:
        _CACHE["nc"] = build_bass()
    nc = _CACHE["nc"]

    in_maps = make_core_inputs(
        np.asarray(x, np.float32), np.asarray(W_q, np.float32),
        np.asarray(W_k, np.float32), np.asarray(W_v, np.float32),
        np.asarray(W_o, np.float32))

    res = run_bass_kernel_spmd(nc, in_maps, core_ids=list(range(NCORES)), trace=TRACE)
    LAST_RESULTS = res

    out = np.zeros((C, E), np.float32)
    for r in res.results:
        out += r["out_part"]
    return out.reshape(B, C, E)


# revision 15
# speedup vs baseline: 1.1241x; 1.1241x over previous
"""Trainium2 Bass kernel for GQA MultiHeadAttention (nn_MultiHeadAttention_74028056314029).

Reference computation (fp32, single device):
    Q = x @ W_q.T         [C, H*D]   (H=32 query heads)
    K = x @ W_k.T         [C, KV*D]  (KV=8 kv heads, GQA group G=4)
    V = x @ W_v.T
    per query head: softmax(causal(Q_h K_h^T / sqrt(D))) @ V_h
    out = hidden @ W_o.T  [C, E]

Sharding (8 NeuronCores, tensor-parallel over heads):
    core c owns query heads [4c, 4c+4) == KV group c (1 KV head).
    W_q/W_o are split on the H*D dim, W_k/W_v on the KV*D dim.
    Each core computes a full-shape partial of the output projection;
    the partials are summed on the host (no on-device collective needed).

Device scheme (all matmuls bf16, fp32 PSUM accumulation):
    - host pre-transposes x and the weight shards so every DMA is contiguous
    - Q.T/K.T live [head_dim, seq] so scores come out transposed [s, q];
      two heads' score matmuls run concurrently in the PE array (row tiling,
      K=64 each at partition offsets 0/64)
    - exp on ACT with the 1/sqrt(D) scale fused; causal diagonal masked by a
      bf16 upper-triangular multiply
    - attn@V appends a ones column to V so the softmax denominator drops out
      of the same accumulation (row 64); 1/den computed as exp(-ln(den)) on
      ACT (the DVE reciprocal is ~7x slower per element) and broadcast across
      partitions with a K=1 PE matmul; the multiply is deferred off the PE
      critical path so the PE never idles into a HAM re-throttle
"""

import os
import numpy as np
import ml_dtypes

E, H, KVH, D = 2048, 32, 8, 64
B, C = 1, 2048
G = H // KVH              # 4 query heads per core
NCORES = 8
HD_C = G * D              # 256 query head dims per core
P = 128
NE = E // P               # 16 contraction chunks
NQ = C // P               # 16 sequence chunks
SW = 512                  # strip width (one PSUM bank of fp32)
NS = C // SW              # 4 strips

BF16 = ml_dtypes.bfloat16

_CACHE: dict = {}
LAST_RESULTS = None       # BassKernelResults of the most recent run (for profiling)
TRACE = bool(int(os.environ.get("KERNEL_TRACE", "0")))


def build_bass():
    import concourse.tile as tile
    import concourse.mybir as mybir
    from concourse import bacc
    from concourse.masks import make_identity

    bf16 = mybir.dt.bfloat16
    f32 = mybir.dt.float32
    AF = mybir.ActivationFunctionType

    nc = bacc.Bacc()
    xT = nc.declare_dram_parameter("xT", [E, C], bf16, isOutput=False)
    wqT = nc.declare_dram_parameter("wqT", [E, HD_C], bf16, isOutput=False)
    wkvT = nc.declare_dram_parameter("wkvT", [E, 2 * D], bf16, isOutput=False)
    woT = nc.declare_dram_parameter("woT", [HD_C, E], bf16, isOutput=False)
    tri = nc.declare_dram_parameter("tri", [P, P], bf16, isOutput=False)
    outp = nc.declare_dram_parameter("out_part", [C, E], f32, isOutput=True)

    with tile.TileContext(nc) as tc:
        with (
            tc.tile_pool(name="big", bufs=1) as big,
            tc.tile_pool(name="expp", bufs=6) as expp,
            tc.tile_pool(name="lnp", bufs=2) as lnp,
            tc.tile_pool(name="recp", bufs=3) as recp,
            tc.tile_pool(name="htp", bufs=2) as htp,
            tc.tile_pool(name="outs", bufs=2) as outs,
            tc.tile_pool(name="ps", bufs=4, space="PSUM") as ps,
            tc.tile_pool(name="psx", bufs=2, space="PSUM") as psx,
            tc.tile_pool(name="psh", bufs=2, space="PSUM") as psh,
        ):
            # ---- persistent SBUF tensors ----
            x_sb = big.tile([P, NE, C], bf16)        # x.T: E on partitions
            wq_sb = big.tile([P, NE, HD_C], bf16)
            wkv_sb = big.tile([P, NE, 2 * D], bf16)  # [W_k | W_v] shard, transposed
            wo_sb = big.tile([P, 2, E], bf16)        # W_o shard transposed: hd on partitions
            tri_sb = big.tile([P, P], bf16)          # upper-tri ones (q>=s valid)
            ident = big.tile([P, P], bf16)
            ones_sb = big.tile([P, D], bf16)         # ones row for the K=1 PE broadcast
            kt_sb = big.tile([P, C], bf16)           # K.T duplicated on both halves
            vt_sb = big.tile([P, C], bf16)           # V.T staged at partitions 64:128
            v_sb = big.tile([P, NQ, D + 1], bf16)    # V natural + ones column
            qt_sb = big.tile([P, 2, C], bf16)        # Q.T: head-dim on partitions
            hid_sb = big.tile([P, 2, C], bf16)       # hidden.T (raw, then normalized)

            xTr = xT[:].rearrange("(eo p) c -> p eo c", p=P)
            for eo in range(NE):     # per-chunk streams so matmuls start early
                nc.sync.dma_start(out=x_sb[:, eo, :], in_=xTr[:, eo, :])
            nc.sync.dma_start(out=wq_sb, in_=wqT[:].rearrange("(eo p) m -> p eo m", p=P))
            nc.sync.dma_start(out=wkv_sb, in_=wkvT[:].rearrange("(eo p) m -> p eo m", p=P))
            nc.sync.dma_start(out=wo_sb, in_=woT[:].rearrange("(ho p) e -> p ho e", p=P))
            nc.sync.dma_start(out=tri_sb, in_=tri[:])
            make_identity(nc, ident)
            nc.vector.memset(v_sb, 1.0)   # ones column survives; V copies overwrite the rest
            nc.vector.memset(ones_sb, 1.0)

            # ---- K/V projection: psum = [K.T ; V.T]; eo-outer so x streams ----
            pkv = [ps.tile([P, SW], f32, tag="mm", name=f"pkv{s}") for s in range(NS)]
            for eo in range(NE):
                for s in range(NS):
                    nc.tensor.matmul(
                        pkv[s], wkv_sb[:, eo, :], x_sb[:, eo, s * SW:(s + 1) * SW],
                        start=(eo == 0), stop=(eo == NE - 1))
            for s in range(NS):
                nc.vector.tensor_copy(out=kt_sb[0:D, s * SW:(s + 1) * SW], in_=pkv[s][0:D, :])
                nc.vector.tensor_copy(out=vt_sb[D:P, s * SW:(s + 1) * SW], in_=pkv[s][D:P, :])
            # duplicate K.T to partitions 64:128 (for row-tiled odd heads)
            nc.sync.dma_start(out=kt_sb[D:P, :], in_=kt_sb[0:D, :])
            # transpose V.T -> V natural [s, d] chunks
            for i in range(NQ):
                ptr = psx.tile([P, D], bf16, tag="aux")
                nc.tensor.transpose(ptr, vt_sb[D:P, i * P:(i + 1) * P], ident[D:P, D:P])
                nc.vector.tensor_copy(out=v_sb[:, i, 0:D], in_=ptr)

            # ---- Q projection (Q.T layout: head dim on partitions) ----
            for m in range(2):
                pq = [ps.tile([P, SW], f32, tag="mm", name=f"pq{s}") for s in range(NS)]
                for eo in range(NE):
                    for s in range(NS):
                        nc.tensor.matmul(
                            pq[s], wq_sb[:, eo, m * P:(m + 1) * P],
                            x_sb[:, eo, s * SW:(s + 1) * SW],
                            start=(eo == 0), stop=(eo == NE - 1))
                for s in range(NS):
                    nc.vector.tensor_copy(out=qt_sb[:, m, s * SW:(s + 1) * SW], in_=pq[s])

            # ---- attention: head pairs run their score matmuls concurrently ----
            ln_tiles = {}

            def emit_attention(pair):
                m = pair
                ln = {h: lnp.tile([P, C], f32, tag="ln", name=f"ln{pair}_{h}") for h in (0, 1)}
                ln_tiles[2 * pair] = ln[0]
                ln_tiles[2 * pair + 1] = ln[1]
                for j in range(NS):
                    ph = {h: psh.tile([D + 1, SW], f32, tag="hid", name=f"ph{pair}_{j}_{h}") for h in (0, 1)}
                    for i in range(4 * j + 4):
                        qlo = max(SW * j, P * i)
                        qhi = SW * (j + 1)
                        w = qhi - qlo
                        llo = qlo - SW * j
                        for h in (0, 1):      # row-tiled pair: K=64 at offsets 0/64
                            poff = D * h
                            psc = ps.tile([P, SW], f32, tag="mm")
                            nc.tensor.matmul(
                                psc[:, :w],
                                kt_sb[poff:poff + D, i * P:(i + 1) * P],
                                qt_sb[poff:poff + D, m, qlo:qhi],
                                start=True, stop=True)
                            et = expp.tile([P, SW], bf16, tag="exp")
                            nc.scalar.activation(out=et[:, :w], in_=psc[:, :w],
                                                 func=AF.Exp, scale=0.125)
                            if qlo == P * i:   # diagonal block: zero q<s entries
                                nc.vector.tensor_mul(et[:, 0:P], et[:, 0:P], tri_sb)
                            nc.tensor.matmul(
                                ph[h][:, llo:], v_sb[:, i, :], et[:, :w],
                                start=(i == 0), stop=(i == 4 * j + 3),
                                skip_group_check=True)
                    sl = slice(SW * j, SW * (j + 1))
                    for h in (0, 1):
                        # move raw hidden + ln(den) out of PSUM so the strip
                        # tiles recycle quickly
                        if h == 0:
                            nc.vector.tensor_copy(out=hid_sb[0:D, m, sl], in_=ph[h][0:D, :])
                        else:
                            nc.vector.tensor_copy(out=ht_tiles[2 * pair + 1][:, sl],
                                                  in_=ph[h][0:D, :])
                        nc.scalar.activation(out=ln[h][D:D + 1, sl], in_=ph[h][D:D + 1, :],
                                             func=AF.Ln)
                # 1/den = exp(-ln(den)), bf16 is plenty for a softmax normalizer
                for h in (0, 1):
                    rec = recp.tile([P, C], bf16, tag="rec")
                    rec_tiles[2 * pair + h] = rec
                    nc.scalar.activation(out=rec[D:D + 1, :], in_=ln[h][D:D + 1, :],
                                         func=AF.Exp, scale=-1.0)

            def emit_normalize(hh):
                m, odd = hh // 2, hh % 2
                rec = rec_tiles[hh]
                for s in range(NS):
                    sl = slice(s * SW, (s + 1) * SW)
                    pb = psx.tile([D, SW], f32, tag="aux")
                    nc.tensor.matmul(pb, ones_sb[D:D + 1, :], rec[D:D + 1, sl],
                                     start=True, stop=True)
                    if not odd:
                        nc.vector.tensor_mul(hid_sb[0:D, m, sl], hid_sb[0:D, m, sl], pb)
                    else:
                        ht = ht_tiles[hh]
                        nc.vector.tensor_mul(ht[:, sl], ht[:, sl], pb)
                if odd:
                    nc.sync.dma_start(out=hid_sb[D:P, m, :], in_=ht_tiles[hh])

            rec_tiles = {}
            ht_tiles = {1: htp.tile([D, C], bf16, tag="ht", name="ht1"),
                        3: htp.tile([D, C], bf16, tag="ht", name="ht3")}
            emit_attention(0)
            emit_attention(1)
            for hh in range(4):
                emit_normalize(hh)

            # ---- output projection partial: out[q, e] = hidden @ W_o_c.T ----
            for qc in range(NQ):
                o_sb = outs.tile([P, E], f32, tag="o")
                po = [ps.tile([P, SW], f32, tag="mm", name=f"po{es}") for es in range(NS)]
                for m in range(2):
                    for es in range(NS):
                        nc.tensor.matmul(
                            po[es], hid_sb[:, m, qc * P:(qc + 1) * P],
                            wo_sb[:, m, es * SW:(es + 1) * SW],
                            start=(m == 0), stop=(m == 1))
                for es in range(NS):
                    nc.vector.tensor_copy(out=o_sb[:, es * SW:(es + 1) * SW], in_=po[es])
                nc.sync.dma_start(out=outp[qc * P:(qc + 1) * P, :], in_=o_sb)

    nc.finalize()
    return nc


def make_core_inputs(x, W_q, W_k, W_v, W_o):
    """Host-side shard + pre-transpose + bf16 cast. Returns list of in_maps."""
    x2 = np.ascontiguousarray(x.reshape(C, E).T).astype(BF16)      # [E, C]
    tri_np = np.triu(np.ones((P, P), np.float32)).astype(BF16)     # q>=s valid
    in_maps = []
    for c in range(NCORES):
        qsl = slice(c * HD_C, (c + 1) * HD_C)
        ksl = slice(c * D, (c + 1) * D)
        wq_t = np.ascontiguousarray(W_q[qsl].T).astype(BF16)                    # [E, 256]
        wkv = np.concatenate([W_k[ksl], W_v[ksl]], axis=0)                      # [128, E]
        wkv_t = np.ascontiguousarray(wkv.T).astype(BF16)                        # [E, 128]
        wo_t = np.ascontiguousarray(W_o[:, qsl].T).astype(BF16)                 # [256, E]
        in_maps.append({
            "xT": x2, "wqT": wq_t, "wkvT": wkv_t, "woT": wo_t, "tri": tri_np,
        })
    return in_maps


def kernel(x, W_q, W_k, W_v, W_o):
    global LAST_RESULTS
    from concourse.bass_utils import run_bass_kernel_spmd

    if "nc" not in _CACHE:
        _CACHE["nc"] = build_bass()
    nc = _CACHE["nc"]

    in_maps = make_core_inputs(
        np.asarray(x, np.float32), np.asarray(W_q, np.float32),
        np.asarray(W_k, np.float32), np.asarray(W_v, np.float32),
        np.asarray(W_o, np.float32))

    res = run_bass_kernel_spmd(nc, in_maps, core_ids=list(range(NCORES)), trace=TRACE)
    LAST_RESULTS = res

    out = np.zeros((C, E), np.float32)
    for r in res.results:
        out += r["out_part"]
    return out.reshape(B, C, E)


# revision 16
# speedup vs baseline: 1.2099x; 1.0763x over previous
"""Trainium2 Bass kernel for GQA MultiHeadAttention (nn_MultiHeadAttention_74028056314029).

Reference computation (fp32, single device):
    Q = x @ W_q.T         [C, H*D]   (H=32 query heads)
    K = x @ W_k.T         [C, KV*D]  (KV=8 kv heads, GQA group G=4)
    V = x @ W_v.T
    per query head: softmax(causal(Q_h K_h^T / sqrt(D))) @ V_h
    out = hidden @ W_o.T  [C, E]

Sharding (8 NeuronCores, tensor-parallel over heads):
    core c owns query heads [4c, 4c+4) == KV group c (1 KV head).
    W_q/W_o are split on the H*D dim, W_k/W_v on the KV*D dim.
    Each core computes a full-shape partial of the output projection;
    the partials are summed on the host (no on-device collective needed).

Device scheme (all matmuls bf16, fp32 PSUM accumulation):
    - host pre-transposes x and the weight shards so every DMA is contiguous
    - Q.T/K.T live [head_dim, seq] so scores come out transposed [s, q];
      two heads' score matmuls run concurrently in the PE array (row tiling,
      K=64 each at partition offsets 0/64)
    - exp on ACT with the 1/sqrt(D) scale fused; causal diagonal masked by a
      bf16 upper-triangular multiply
    - attn@V appends a ones column to V so the softmax denominator drops out
      of the same accumulation (row 64); 1/den computed as exp(-ln(den)) on
      ACT (the DVE reciprocal is ~7x slower per element) and broadcast across
      partitions with a K=1 PE matmul; the multiply is deferred off the PE
      critical path so the PE never idles into a HAM re-throttle
"""

import os
import numpy as np
import ml_dtypes

E, H, KVH, D = 2048, 32, 8, 64
B, C = 1, 2048
G = H // KVH              # 4 query heads per core
NCORES = 8
HD_C = G * D              # 256 query head dims per core
P = 128
NE = E // P               # 16 contraction chunks
NQ = C // P               # 16 sequence chunks
SW = 512                  # strip width (one PSUM bank of fp32)
NS = C // SW              # 4 strips

BF16 = ml_dtypes.bfloat16

_CACHE: dict = {}
LAST_RESULTS = None       # BassKernelResults of the most recent run (for profiling)
TRACE = bool(int(os.environ.get("KERNEL_TRACE", "0")))


def build_bass():
    import concourse.tile as tile
    import concourse.mybir as mybir
    from concourse import bacc
    from concourse.masks import make_identity

    bf16 = mybir.dt.bfloat16
    f32 = mybir.dt.float32
    AF = mybir.ActivationFunctionType

    nc = bacc.Bacc()
    xT = nc.declare_dram_parameter("xT", [E, C], bf16, isOutput=False)
    wqT = nc.declare_dram_parameter("wqT", [E, HD_C], bf16, isOutput=False)
    wkvT = nc.declare_dram_parameter("wkvT", [E, 2 * D], bf16, isOutput=False)
    woT = nc.declare_dram_parameter("woT", [HD_C, E], bf16, isOutput=False)
    tri = nc.declare_dram_parameter("tri", [P, P], bf16, isOutput=False)
    outp = nc.declare_dram_parameter("out_part", [C, E], f32, isOutput=True)
    scr_den = nc.dram_tensor("scr_den", [G, 1, C], f32)
    scr_rec = nc.dram_tensor("scr_rec", [G, 1, C], f32)

    with tile.TileContext(nc) as tc:
        with (
            tc.tile_pool(name="big", bufs=1) as big,
            tc.tile_pool(name="expp", bufs=6) as expp,
            tc.tile_pool(name="denp", bufs=2) as denp,
            tc.tile_pool(name="d128", bufs=2) as d128p,
            tc.tile_pool(name="recp", bufs=3) as recp,
            tc.tile_pool(name="htp", bufs=2) as htp,
            tc.tile_pool(name="outs", bufs=2) as outs,
            tc.tile_pool(name="ps", bufs=4, space="PSUM") as ps,
            tc.tile_pool(name="psx", bufs=2, space="PSUM") as psx,
            tc.tile_pool(name="psh", bufs=2, space="PSUM") as psh,
        ):
            # ---- persistent SBUF tensors ----
            x_sb = big.tile([P, NE, C], bf16)        # x.T: E on partitions
            wq_sb = big.tile([P, NE, HD_C], bf16)
            wkv_sb = big.tile([P, NE, 2 * D], bf16)  # [W_k | W_v] shard, transposed
            wo_sb = big.tile([P, 2, E], bf16)        # W_o shard transposed: hd on partitions
            tri_sb = big.tile([P, P], bf16)          # upper-tri ones (q>=s valid)
            ident = big.tile([P, P], bf16)
            ones_sb = big.tile([P, D], bf16)         # ones row for the K=1 PE broadcast
            kt_sb = big.tile([P, C], bf16)           # K.T duplicated on both halves
            vt_sb = big.tile([P, C], bf16)           # V.T staged at partitions 64:128
            v_sb = big.tile([P, NQ, D + 1], bf16)    # V natural + ones column
            qt_sb = big.tile([P, 2, C], bf16)        # Q.T: head-dim on partitions
            hid_sb = big.tile([P, 2, C], bf16)       # hidden.T (raw, then normalized)

            xTr = xT[:].rearrange("(eo p) c -> p eo c", p=P)
            for eo in range(NE):     # per-chunk streams so matmuls start early
                nc.sync.dma_start(out=x_sb[:, eo, :], in_=xTr[:, eo, :])
            nc.sync.dma_start(out=wq_sb, in_=wqT[:].rearrange("(eo p) m -> p eo m", p=P))
            nc.sync.dma_start(out=wkv_sb, in_=wkvT[:].rearrange("(eo p) m -> p eo m", p=P))
            nc.sync.dma_start(out=wo_sb, in_=woT[:].rearrange("(ho p) e -> p ho e", p=P))
            nc.sync.dma_start(out=tri_sb, in_=tri[:])
            make_identity(nc, ident)
            nc.vector.memset(v_sb, 1.0)   # ones column survives; V copies overwrite the rest
            nc.vector.memset(ones_sb, 1.0)

            # ---- K/V projection: psum = [K.T ; V.T]; eo-outer so x streams ----
            pkv = [ps.tile([P, SW], f32, tag="mm", name=f"pkv{s}") for s in range(NS)]
            for eo in range(NE):
                for s in range(NS):
                    nc.tensor.matmul(
                        pkv[s], wkv_sb[:, eo, :], x_sb[:, eo, s * SW:(s + 1) * SW],
                        start=(eo == 0), stop=(eo == NE - 1))
            for s in range(NS):
                nc.vector.tensor_copy(out=kt_sb[0:D, s * SW:(s + 1) * SW], in_=pkv[s][0:D, :])
                nc.vector.tensor_copy(out=vt_sb[D:P, s * SW:(s + 1) * SW], in_=pkv[s][D:P, :])
            # duplicate K.T to partitions 64:128 (for row-tiled odd heads)
            nc.sync.dma_start(out=kt_sb[D:P, :], in_=kt_sb[0:D, :])
            # transpose V.T -> V natural [s, d] chunks
            for i in range(NQ):
                ptr = psx.tile([P, D], bf16, tag="aux")
                nc.tensor.transpose(ptr, vt_sb[D:P, i * P:(i + 1) * P], ident[D:P, D:P])
                nc.vector.tensor_copy(out=v_sb[:, i, 0:D], in_=ptr)

            # ---- Q projection (Q.T layout: head dim on partitions) ----
            for m in range(2):
                pq = [ps.tile([P, SW], f32, tag="mm", name=f"pq{s}") for s in range(NS)]
                for eo in range(NE):
                    for s in range(NS):
                        nc.tensor.matmul(
                            pq[s], wq_sb[:, eo, m * P:(m + 1) * P],
                            x_sb[:, eo, s * SW:(s + 1) * SW],
                            start=(eo == 0), stop=(eo == NE - 1))
                for s in range(NS):
                    nc.vector.tensor_copy(out=qt_sb[:, m, s * SW:(s + 1) * SW], in_=pq[s])

            # ---- attention: head pairs run their score matmuls concurrently ----

            def emit_attention(pair):
                m = pair
                den = {h: denp.tile([P, C], f32, tag="den", name=f"den{pair}_{h}")
                       for h in (0, 1)}
                for j in range(NS):
                    ph = {h: psh.tile([D + 1, SW], f32, tag="hid", name=f"ph{pair}_{j}_{h}") for h in (0, 1)}
                    for i in range(4 * j + 4):
                        qlo = max(SW * j, P * i)
                        qhi = SW * (j + 1)
                        w = qhi - qlo
                        llo = qlo - SW * j
                        for h in (0, 1):      # row-tiled pair: K=64 at offsets 0/64
                            poff = D * h
                            psc = ps.tile([P, SW], f32, tag="mm")
                            nc.tensor.matmul(
                                psc[:, :w],
                                kt_sb[poff:poff + D, i * P:(i + 1) * P],
                                qt_sb[poff:poff + D, m, qlo:qhi],
                                start=True, stop=True)
                            et = expp.tile([P, SW], bf16, tag="exp")
                            nc.scalar.activation(out=et[:, :w], in_=psc[:, :w],
                                                 func=AF.Exp, scale=0.125)
                            if qlo == P * i:   # diagonal block: zero q<s entries
                                nc.vector.tensor_mul(et[:, 0:P], et[:, 0:P], tri_sb)
                            nc.tensor.matmul(
                                ph[h][:, llo:], v_sb[:, i, :], et[:, :w],
                                start=(i == 0), stop=(i == 4 * j + 3),
                                skip_group_check=True)
                    sl = slice(SW * j, SW * (j + 1))
                    for h in (0, 1):
                        # move raw hidden + den out of PSUM so the strip
                        # tiles recycle quickly
                        if h == 0:
                            nc.vector.tensor_copy(out=hid_sb[0:D, m, sl], in_=ph[h][0:D, :])
                        else:
                            nc.vector.tensor_copy(out=ht_tiles[2 * pair + 1][:, sl],
                                                  in_=ph[h][0:D, :])
                        nc.vector.tensor_copy(out=den[h][D:D + 1, sl], in_=ph[h][D:D + 1, :])
                # 1/den: bounce through DRAM to reshape the 1-lane row into
                # [128, 16] so the DVE reciprocal is free-size-cheap
                for h in (0, 1):
                    hh = 2 * pair + h
                    nc.sync.dma_start(out=scr_den[hh], in_=den[h][D:D + 1, :])
                    dd = d128p.tile([P, NQ], f32, tag="d128", name=f"dd{hh}")
                    nc.sync.dma_start(out=dd, in_=scr_den[hh].rearrange("a (p o) -> (a p) o", p=P))
                    rr = d128p.tile([P, NQ], f32, tag="r128", name=f"rr{hh}")
                    nc.vector.reciprocal(out=rr, in_=dd)
                    nc.sync.dma_start(out=scr_rec[hh].rearrange("a (p o) -> (a p) o", p=P), in_=rr)
                    rec = recp.tile([P, C], bf16, tag="rec", name=f"rec{hh}")
                    rec_tiles[hh] = rec
                    nc.gpsimd.dma_start(out=rec[D:D + 1, :], in_=scr_rec[hh])

            def emit_normalize(hh):
                m, odd = hh // 2, hh % 2
                rec = rec_tiles[hh]
                for s in range(NS):
                    sl = slice(s * SW, (s + 1) * SW)
                    pb = psx.tile([D, SW], f32, tag="aux")
                    nc.tensor.matmul(pb, ones_sb[D:D + 1, :], rec[D:D + 1, sl],
                                     start=True, stop=True)
                    if not odd:
                        nc.vector.tensor_mul(hid_sb[0:D, m, sl], hid_sb[0:D, m, sl], pb)
                    else:
                        ht = ht_tiles[hh]
                        nc.vector.tensor_mul(ht[:, sl], ht[:, sl], pb)
                if odd:
                    nc.sync.dma_start(out=hid_sb[D:P, m, :], in_=ht_tiles[hh])

            rec_tiles = {}
            ht_tiles = {1: htp.tile([D, C], bf16, tag="ht", name="ht1"),
                        3: htp.tile([D, C], bf16, tag="ht", name="ht3")}
            emit_attention(0)
            emit_attention(1)
            for hh in range(4):
                emit_normalize(hh)

            # ---- output projection partial: out[q, e] = hidden @ W_o_c.T ----
            for qc in range(NQ):
                o_sb = outs.tile([P, E], f32, tag="o")
                po = [ps.tile([P, SW], f32, tag="mm", name=f"po{es}") for es in range(NS)]
                for m in range(2):
                    for es in range(NS):
                        nc.tensor.matmul(
                            po[es], hid_sb[:, m, qc * P:(qc + 1) * P],
                            wo_sb[:, m, es * SW:(es + 1) * SW],
                            start=(m == 0), stop=(m == 1))
                for es in range(NS):
                    nc.vector.tensor_copy(out=o_sb[:, es * SW:(es + 1) * SW], in_=po[es])
                nc.sync.dma_start(out=outp[qc * P:(qc + 1) * P, :], in_=o_sb)

    nc.finalize()
    return nc


def make_core_inputs(x, W_q, W_k, W_v, W_o):
    """Host-side shard + pre-transpose + bf16 cast. Returns list of in_maps."""
    x2 = np.ascontiguousarray(x.reshape(C, E).T).astype(BF16)      # [E, C]
    tri_np = np.triu(np.ones((P, P), np.float32)).astype(BF16)     # q>=s valid
    in_maps = []
    for c in range(NCORES):
        qsl = slice(c * HD_C, (c + 1) * HD_C)
        ksl = slice(c * D, (c + 1) * D)
        wq_t = np.ascontiguousarray(W_q[qsl].T).astype(BF16)                    # [E, 256]
        wkv = np.concatenate([W_k[ksl], W_v[ksl]], axis=0)                      # [128, E]
        wkv_t = np.ascontiguousarray(wkv.T).astype(BF16)                        # [E, 128]
        wo_t = np.ascontiguousarray(W_o[:, qsl].T).astype(BF16)                 # [256, E]
        in_maps.append({
            "xT": x2, "wqT": wq_t, "wkvT": wkv_t, "woT": wo_t, "tri": tri_np,
        })
    return in_maps


def kernel(x, W_q, W_k, W_v, W_o):
    global LAST_RESULTS
    from concourse.bass_utils import run_bass_kernel_spmd

    if "nc" not in _CACHE:
        _CACHE["nc"] = build_bass()
    nc = _CACHE["nc"]

    in_maps = make_core_inputs(
        np.asarray(x, np.float32), np.asarray(W_q, np.float32),
        np.asarray(W_k, np.float32), np.asarray(W_v, np.float32),
        np.asarray(W_o, np.float32))

    res = run_bass_kernel_spmd(nc, in_maps, core_ids=list(range(NCORES)), trace=TRACE)
    LAST_RESULTS = res

    out = np.zeros((C, E), np.float32)
    for r in res.results:
        out += r["out_part"]
    return out.reshape(B, C, E)
